# revision 16
# baseline (speedup 1.0000x reference)
"""Single-head causal attention on 8 TRN2 NeuronCores.

Problem: x [8, 2048, 1024] f32, Wq/Wk/Wv [1024, 64] f32.
  q = x @ Wq ; k = x @ Wk ; v = x @ Wv        (per batch)
  out = softmax(causal(q k^T / 8)) @ v        [8, 2048, 64]

Sharding: data-parallel over batch -- core i handles batch element i.
No collectives needed.

Per-core kernel (bf16 compute, f32 accumulate), 128-token-tile pipeline:
  1. W loads use the natural row-contiguous layout (2KB descriptors, no
     sub-512B DMA penalty); the d-contraction is chunked INTERLEAVED
     (chunk a = {d : d = 8p + a}) so the natural layout needs no
     on-chip weight transpose -- gpsimd packs [Wq|Wk|Wv] to bf16.
  2. x streams per 256-token group; each 128-tile is cast f32->bf16 on
     DVE directly into the interleaved layout, transposed on the PE
     (8x [128,128] identity matmuls -> PSUM bf16), and copied to the
     x^T SBUF pool by DVE.
  3. Projections are x-stationary: lhsT = x^T tile-chunk, moving
     rhs = [Wq|Wk|Wv] (192 wide) -> PSUM [t,192] in 8 matmuls/tile
     (1536 PE cycles vs 2048 for the W-stationary form).
  4. Q,K land t-major; one DMA-xbar transpose per tile ([t,128] ->
     [qk,t], 8 ucode tiles = ~112ns of DMA) yields Q^T/K^T rows with
     h on partitions. V stays t-major (what PV wants) and is copied
     into V_aug with a ones column (softmax denominator for free).
  5. Attention per 512-row q-block: S^T[tk,tq] = K^T_tile.T @ Q^T
     (contraction h); exp on ACT in k-tile PAIRS (halves the ~185ns
     per-instruction access-latency overhead); causal diagonal via a
     multiplicative 0/1 bf16 mask on DVE; PV accumulates
     out^T[65,tq] += V_aug.T @ P^T in PSUM, row 64 = denominators.
  6. Output: PSUM -> bf16 SBUF copy (gpsimd), PE-transpose back to
     [tq,65], reciprocal-rescale (DVE recip + gpsimd scale), bf16
     store (f32 upcast happens host-side after gather).

Engine budget: PE ~76k cycles (transposes 16.4k, proj 24.6k, S 16.9k,
PV 16.9k, out 1k) is the critical resource; DMA ~29us (x 23.3 =
roofline, W 2.2, qk-xbar 1.8, stores 1.5); ACT owns exp (~20us); DVE
casts/copies/masks (~22us); gpsimd does the PSUM drains (~11us).
"""

import numpy as np

import concourse.bass as bass
import concourse.tile as tile
from concourse import bacc, mybir
from concourse.bass_utils import run_bass_kernel_spmd

B, T, D, H = 8, 2048, 1024, 64
P = 128            # partitions / tile edge
ND = D // P        # 8 d-chunks (interleaved: chunk a = {d : d = 8p + a})
NT = T // P        # 16 token tiles
NB = T // 512      # 4 q-blocks of 512 rows
VA = 80            # v_aug padded k-tile stride

FP32 = mybir.dt.float32
BF16 = mybir.dt.bfloat16

_compiled = None


def _build():
    nc = bacc.Bacc("TRN2", target_bir_lowering=False, debug=False, num_devices=8)

    x_d = nc.dram_tensor("x", [T, D], FP32, kind="ExternalInput").ap()
    wq_d = nc.dram_tensor("Wq", [D, H], FP32, kind="ExternalInput").ap()
    wk_d = nc.dram_tensor("Wk", [D, H], FP32, kind="ExternalInput").ap()
    wv_d = nc.dram_tensor("Wv", [D, H], FP32, kind="ExternalInput").ap()
    out_d = nc.dram_tensor("out", [T, H], BF16, kind="ExternalOutput").ap()

    with tile.TileContext(nc) as tc:
        _kernel(tc, out_d, x_d, wq_d, wk_d, wv_d)

    nc.compile()
    return nc


def _kernel(tc, out_d, x_d, wq_d, wk_d, wv_d):
    nc = tc.nc
    from contextlib import ExitStack

    ctx = ExitStack()
    with ctx:
        const = ctx.enter_context(tc.tile_pool(name="const", bufs=1))
        wstage = ctx.enter_context(tc.tile_pool(name="wstage", bufs=3))
        xload = ctx.enter_context(tc.tile_pool(name="xload", bufs=3))
        xbtp = ctx.enter_context(tc.tile_pool(name="xbtp", bufs=3))
        xtp = ctx.enter_context(tc.tile_pool(name="xtp", bufs=1))
        qkp = ctx.enter_context(tc.tile_pool(name="qkp", bufs=1))
        qksp = ctx.enter_context(tc.tile_pool(name="qksp", bufs=2))
        vsb = ctx.enter_context(tc.tile_pool(name="vsb", bufs=1))
        ptp = ctx.enter_context(tc.tile_pool(name="ptp", bufs=3))
        obp = ctx.enter_context(tc.tile_pool(name="obp", bufs=2))
        osbp = ctx.enter_context(tc.tile_pool(name="osbp", bufs=2))
        recp = ctx.enter_context(tc.tile_pool(name="recp", bufs=2))
        pxt = ctx.enter_context(tc.tile_pool(name="pxt", bufs=1, space="PSUM"))
        psS = ctx.enter_context(tc.tile_pool(name="psS", bufs=2, space="PSUM"))
        pout = ctx.enter_context(tc.tile_pool(name="pout", bufs=1, space="PSUM"))
        psmall = ctx.enter_context(tc.tile_pool(name="psmall", bufs=2, space="PSUM"))

        # ---- constants ----
        ident_bf = const.tile([P, P], BF16)
        from concourse.masks import make_identity
        make_identity(nc, ident_bf[:])

        # 0/1 upper-triangular (incl. diagonal) bf16 mask in [tk, tq]
        # orientation: valid when tq >= tk (col >= row).
        tri01 = const.tile([P, P], BF16)
        nc.gpsimd.memset(tri01[:], 1.0)
        nc.gpsimd.affine_select(
            out=tri01[:], in_=tri01[:],
            compare_op=mybir.AluOpType.is_ge,
            fill=0.0, base=0,
            pattern=[[1, P]], channel_multiplier=-1)

        # V_aug [t-part, k-tile, 80]: col 64 = 1.0 (denominator row).
        v_aug = vsb.tile([P, NT, VA], BF16)
        nc.gpsimd.memset(v_aug[:, :, H:H + 1], 1.0)

        # ---- weight loads (natural layout; interleaved d-chunking) ----
        # Partition p holds rows d = 8p..8p+7 as one contiguous 2KB
        # descriptor; chunk a of the contraction is the partition-slice
        # [:, a, :], so no weight transpose is ever needed.
        w_all = const.tile([P, ND, 3 * H], BF16)   # [Wq | Wk | Wv] per slot
        wnats = []
        for w_dram, name in ((wq_d, "wq"), (wk_d, "wk"), (wv_d, "wv")):
            wn = wstage.tile([P, ND, H], FP32, tag="wstage", name=f"stg_{name}")
            nc.sync.dma_start(out=wn[:], in_=w_dram.rearrange(
                "(p a) h -> p a h", p=P))
            wnats.append(wn)

        # ---- x loads: 8 groups of 2 tiles ----
        x_r = x_d.rearrange("(g u p) d -> g p u d", p=P, u=2)
        xfs = {}
        for g in range(NT // 2):
            xf = xload.tile([P, 2, D], FP32, tag="xf", name=f"xf{g}")
            nc.sync.dma_start(out=xf[:], in_=x_r[g])
            xfs[g] = xf

        # weight pack (gpsimd): cast + concat into w_all
        for j, wn in enumerate(wnats):
            nc.gpsimd.tensor_copy(out=w_all[:, :, j * H:(j + 1) * H], in_=wn[:])

        # persistent SBUF state
        xT = xtp.tile([P, ND, T], BF16)      # x^T, interleaved chunks
        # Q^T/K^T with h on partitions 64:128 and ZEROS on 0:64 (the xbar
        # needs 128-col inputs; the zero half contributes nothing to the
        # h-contraction and costs no extra matmul cycles).
        qzT = qkp.tile([P, T], BF16)
        kzT = qkp.tile([P, T], BF16)
        # staging slots: [:, u, 0:64]=0, [:, u, 64:128]=q (u=0) | k (u=1);
        # one merged PSUM->SBUF copy fills both, rotated i%2.
        qk_stage = [const.tile([P, 2, P], BF16, name=f"qkz{s}")
                    for s in range(2)]
        for t_ in qk_stage:
            nc.gpsimd.memset(t_[:, :, 0:H], 0.0)

        # ---- per-tile pipeline ----
        def tile_work(i):
            g, u = divmod(i, 2)
            # cast f32 -> bf16 into the interleaved (a, j) layout:
            # element d of the tile lands at [a = d % 8, j = d // 8].
            xbt = xbtp.tile([P, ND, P], BF16, tag="xbt", name=f"xbt{i}")
            nc.vector.tensor_copy(
                out=xbt[:].rearrange("p a j -> p j a"), in_=xfs[g][:, u, :])
            # PE transposes: chunk a -> x^T[:, a, tile i]
            px = pxt.tile([P, ND, P], BF16, tag="pxt", name=f"px{i}")
            for a in range(ND):
                nc.tensor.transpose(px[:, a, :], xbt[:, a, :], ident_bf[:])
            nc.vector.tensor_copy(
                out=xT[:, :, i * P:(i + 1) * P], in_=px[:])
            # projection: x^T-stationary, W moving (192 wide)
            ps_p = psmall.tile([P, 3 * H], FP32, tag="small", name=f"psp{i}")
            for a in range(ND):
                nc.tensor.matmul(ps_p[:], xT[:, a, i * P:(i + 1) * P],
                                 w_all[:, a, :],
                                 start=(a == 0), stop=(a == ND - 1))
            # Q,K -> half-zero bf16 stages -> xbar transposes
            qkz = qk_stage[i % 2]
            nc.vector.tensor_copy(out=qkz[:, :, H:P], in_=ps_p[:, 0:P])
            nc.scalar.dma_start(out=qzT[:, i * P:(i + 1) * P],
                                in_=qkz[:, 0, :], transpose=True)
            nc.sync.dma_start(out=kzT[:, i * P:(i + 1) * P],
                              in_=qkz[:, 1, :], transpose=True)
            # V stays t-major
            nc.scalar.copy(out=v_aug[:, i, 0:H], in_=ps_p[:, P:P + H])

        # ---- attention ----
        stores = []

        def diag(b, ki):
            return 4 * b <= ki < 4 * b + 4

        def attention_block(b):
            nk = 4 * b + 4
            qlo = 512 * b
            pairs = [(2 * j, 2 * j + 1) for j in range(nk // 2)]
            ps_o = pout.tile([H + 1, 512], FP32, tag="pout", name=f"pso{b}")

            def s_exp(pr):
                k0, k1 = pr
                w0 = max(0, k0 * P - qlo)
                w1 = max(0, k1 * P - qlo)
                ps = psS.tile([P, 1024], FP32, tag="psS", name=f"psS{b}_{k0}")
                pt = ptp.tile([P, 1024], BF16, tag="pt", name=f"pt{b}_{k0}")
                for ki, w, pos in ((k0, w0, 0), (k1, w1, 512)):
                    nc.tensor.matmul(
                        ps[:, pos + w:pos + 512],
                        kzT[:, ki * P:(ki + 1) * P],
                        qzT[:, qlo + w:qlo + 512],
                        start=True, stop=True)
                if b == 0:
                    # fresh PSUM slots: exp only over written regions
                    for ki, w, pos in ((k0, w0, 0), (k1, w1, 512)):
                        nc.scalar.activation(
                            out=pt[:, pos + w:pos + 512],
                            in_=ps[:, pos + w:pos + 512],
                            func=mybir.ActivationFunctionType.Exp,
                            scale=0.125)
                else:
                    # one wide exp; the [512, 512+w1) gap holds stale
                    # (finite) values from an earlier pair and is never
                    # read by PV.
                    nc.scalar.activation(
                        out=pt[:, w0:1024], in_=ps[:, w0:1024],
                        func=mybir.ActivationFunctionType.Exp,
                        scale=0.125)
                for ki, w, pos in ((k0, w0, 0), (k1, w1, 512)):
                    if diag(b, ki):
                        nc.gpsimd.tensor_mul(pt[:, pos + w:pos + w + P],
                                             pt[:, pos + w:pos + w + P],
                                             tri01[:])
                return pt, w0, w1

            def pv(idx, pr, pt_w):
                k0, k1 = pr
                pt, w0, w1 = pt_w
                for ki, w, pos in ((k0, w0, 0), (k1, w1, 512)):
                    nc.tensor.matmul(
                        ps_o[:, w:512], v_aug[:, ki, 0:H + 1],
                        pt[:, pos + w:pos + 512],
                        start=(idx == 0 and ki == k0),
                        stop=(idx == len(pairs) - 1 and ki == k1))

            pending = s_exp(pairs[0])
            for idx, pr in enumerate(pairs):
                nxt = s_exp(pairs[idx + 1]) if idx + 1 < len(pairs) else None
                pv(idx, pr, pending)
                pending = nxt

            # ---- output stage ----
            ob = obp.tile([H + 1, 512], BF16, tag="ob", name=f"ob{b}")
            nc.scalar.copy(out=ob[:], in_=ps_o[:])
            pot = psmall.tile([P, 4, VA], BF16, tag="small", name=f"pot{b}")
            for j in range(4):
                nc.tensor.transpose(pot[:, j, 0:H + 1],
                                    ob[:, j * P:(j + 1) * P],
                                    ident_bf[0:H + 1, 0:H + 1])
            rec = recp.tile([P, 4], FP32, tag="rec", name=f"rec{b}")
            nc.vector.reciprocal(rec[:], pot[:, :, H])
            osb = osbp.tile([P, 4, H], BF16, tag="osb", name=f"osb{b}")
            for j in range(4):
                nc.vector.tensor_scalar_mul(osb[:, j, :], pot[:, j, 0:H],
                                            rec[:, j:j + 1])
            stores.append(
                (out_d.rearrange("(b j p) h -> b p j h", p=P, j=4)[b], osb))

        for i in range(NT):
            tile_work(i)
            if i % 4 == 3:
                attention_block(i // 4)

        for dst, osb in stores:
            nc.sync.dma_start(out=dst, in_=osb[:])


def _run(inputs, trace=False, **kw):
    global _compiled
    if _compiled is None:
        _compiled = _build()
    nc = _compiled
    x = np.ascontiguousarray(inputs["x"], dtype=np.float32)
    wq = np.ascontiguousarray(inputs["Wq"], dtype=np.float32)
    wk = np.ascontiguousarray(inputs["Wk"], dtype=np.float32)
    wv = np.ascontiguousarray(inputs["Wv"], dtype=np.float32)
    in_maps = [
        {"x": np.ascontiguousarray(x[i]), "Wq": wq, "Wk": wk, "Wv": wv}
        for i in range(B)
    ]
    res = run_bass_kernel_spmd(nc, in_maps, core_ids=list(range(B)),
                               trace=trace, **kw)
    out = np.stack(
        [np.asarray(res.results[i]["out"]).astype(np.float32) for i in range(B)],
        axis=0)
    return out, res


def kernel(x, Wq, Wk, Wv):
    out, _ = _run({"x": x, "Wq": Wq, "Wk": Wk, "Wv": Wv})
    return out


# revision 19
# speedup vs baseline: 1.1186x; 1.1186x over previous
"""Single-head causal attention on 8 TRN2 NeuronCores.

Problem: x [8, 2048, 1024] f32, Wq/Wk/Wv [1024, 64] f32.
  q = x @ Wq ; k = x @ Wk ; v = x @ Wv        (per batch)
  out = softmax(causal(q k^T / 8)) @ v        [8, 2048, 64]

Sharding: data-parallel over batch -- core i handles batch element i.
No collectives needed.

Per-core kernel (bf16 compute, f32 accumulate), 128-token-tile pipeline:
  1. W loads use the natural row-contiguous layout (2KB descriptors, no
     sub-512B DMA penalty); the d-contraction is chunked INTERLEAVED
     (chunk a = {d : d = 8p + a}) so the natural layout needs no
     on-chip weight transpose -- gpsimd packs [Wq|Wk|Wv] to bf16.
  2. x streams per 256-token group; each 128-tile is cast f32->bf16 on
     DVE directly into the interleaved layout, transposed on the PE
     (8x [128,128] identity matmuls -> PSUM bf16), and copied to the
     x^T SBUF pool by DVE.
  3. Projections are x-stationary: lhsT = x^T tile-chunk, moving
     rhs = [Wq|Wk|Wv] (192 wide) -> PSUM [t,192] in 8 matmuls/tile
     (1536 PE cycles vs 2048 for the W-stationary form).
  4. Q,K land t-major; one DMA-xbar transpose per tile ([t,128] ->
     [qk,t], 8 ucode tiles = ~112ns of DMA) yields Q^T/K^T rows with
     h on partitions. V stays t-major (what PV wants) and is copied
     into V_aug with a ones column (softmax denominator for free).
  5. Attention per 512-row q-block: S^T[tk,tq] = K^T_tile.T @ Q^T
     (contraction h); exp on ACT in k-tile PAIRS (halves the ~185ns
     per-instruction access-latency overhead); causal diagonal via a
     multiplicative 0/1 bf16 mask on DVE; PV accumulates
     out^T[65,tq] += V_aug.T @ P^T in PSUM, row 64 = denominators.
  6. Output: PSUM -> bf16 SBUF copy (gpsimd), PE-transpose back to
     [tq,65], reciprocal-rescale (DVE recip + gpsimd scale), bf16
     store (f32 upcast happens host-side after gather).

Engine budget: PE ~76k cycles (transposes 16.4k, proj 24.6k, S 16.9k,
PV 16.9k, out 1k) is the critical resource; DMA ~29us (x 23.3 =
roofline, W 2.2, qk-xbar 1.8, stores 1.5); ACT owns exp (~20us); DVE
casts/copies/masks (~22us); gpsimd does the PSUM drains (~11us).
"""

import numpy as np

import concourse.bass as bass
import concourse.tile as tile
from concourse import bacc, mybir
from concourse.bass_utils import run_bass_kernel_spmd

B, T, D, H = 8, 2048, 1024, 64
P = 128            # partitions / tile edge
ND = D // P        # 8 d-chunks (interleaved: chunk a = {d : d = 8p + a})
NT = T // P        # 16 token tiles
NB = T // 512      # 4 q-blocks of 512 rows
VA = 80            # v_aug padded k-tile stride

FP32 = mybir.dt.float32
BF16 = mybir.dt.bfloat16

_compiled = None


def _build():
    nc = bacc.Bacc("TRN2", target_bir_lowering=False, debug=False, num_devices=8)

    x_d = nc.dram_tensor("x", [T, D], FP32, kind="ExternalInput").ap()
    wq_d = nc.dram_tensor("Wq", [D, H], FP32, kind="ExternalInput").ap()
    wk_d = nc.dram_tensor("Wk", [D, H], FP32, kind="ExternalInput").ap()
    wv_d = nc.dram_tensor("Wv", [D, H], FP32, kind="ExternalInput").ap()
    out_d = nc.dram_tensor("out", [T, H], BF16, kind="ExternalOutput").ap()

    with tile.TileContext(nc) as tc:
        _kernel(tc, out_d, x_d, wq_d, wk_d, wv_d)

    nc.compile()
    return nc


def _kernel(tc, out_d, x_d, wq_d, wk_d, wv_d):
    nc = tc.nc
    from contextlib import ExitStack

    ctx = ExitStack()
    with ctx:
        const = ctx.enter_context(tc.tile_pool(name="const", bufs=1))
        wstage = ctx.enter_context(tc.tile_pool(name="wstage", bufs=3))
        xload = ctx.enter_context(tc.tile_pool(name="xload", bufs=4))
        xbtp = ctx.enter_context(tc.tile_pool(name="xbtp", bufs=3))
        xtp = ctx.enter_context(tc.tile_pool(name="xtp", bufs=1))
        qkp = ctx.enter_context(tc.tile_pool(name="qkp", bufs=1))
        qksp = ctx.enter_context(tc.tile_pool(name="qksp", bufs=2))
        vsb = ctx.enter_context(tc.tile_pool(name="vsb", bufs=1))
        ptp = ctx.enter_context(tc.tile_pool(name="ptp", bufs=3))
        obp = ctx.enter_context(tc.tile_pool(name="obp", bufs=2))
        osbp = ctx.enter_context(tc.tile_pool(name="osbp", bufs=2))
        recp = ctx.enter_context(tc.tile_pool(name="recp", bufs=2))
        pxt = ctx.enter_context(tc.tile_pool(name="pxt", bufs=1, space="PSUM"))
        psS = ctx.enter_context(tc.tile_pool(name="psS", bufs=2, space="PSUM"))
        pout = ctx.enter_context(tc.tile_pool(name="pout", bufs=1, space="PSUM"))
        psmall = ctx.enter_context(tc.tile_pool(name="psmall", bufs=2, space="PSUM"))

        # ---- constants ----
        ident_bf = const.tile([P, P], BF16)
        from concourse.masks import make_identity
        make_identity(nc, ident_bf[:])

        # 0/1 upper-triangular (incl. diagonal) bf16 mask in [tk, tq]
        # orientation: valid when tq >= tk (col >= row).
        tri01 = const.tile([P, P], BF16)
        nc.gpsimd.memset(tri01[:], 1.0)
        nc.gpsimd.affine_select(
            out=tri01[:], in_=tri01[:],
            compare_op=mybir.AluOpType.is_ge,
            fill=0.0, base=0,
            pattern=[[1, P]], channel_multiplier=-1)

        # V_aug [t-part, k-tile, 80]: col 64 = 1.0 (denominator row).
        v_aug = vsb.tile([P, NT, VA], BF16)
        nc.gpsimd.memset(v_aug[:, :, H:H + 1], 1.0)

        # ---- weight loads (natural layout; interleaved d-chunking) ----
        # Partition p holds rows d = 8p..8p+7 as one contiguous 2KB
        # descriptor; chunk a of the contraction is the partition-slice
        # [:, a, :], so no weight transpose is ever needed.
        w_all = const.tile([P, ND, 3 * H], BF16)   # [Wq | Wk | Wv] per slot
        wnats = []
        for w_dram, name in ((wq_d, "wq"), (wk_d, "wk"), (wv_d, "wv")):
            wn = wstage.tile([P, ND, H], FP32, tag="wstage", name=f"stg_{name}")
            nc.sync.dma_start(out=wn[:], in_=w_dram.rearrange(
                "(p a) h -> p a h", p=P))
            wnats.append(wn)

        # ---- x loads: 8 groups of 2 tiles ----
        x_r = x_d.rearrange("(g u p) d -> g p u d", p=P, u=2)
        xfs = {}
        for g in range(NT // 2):
            xf = xload.tile([P, 2, D], FP32, tag="xf", name=f"xf{g}")
            nc.sync.dma_start(out=xf[:], in_=x_r[g])
            xfs[g] = xf

        # weight pack (gpsimd): cast + concat into w_all
        for j, wn in enumerate(wnats):
            nc.gpsimd.tensor_copy(out=w_all[:, :, j * H:(j + 1) * H], in_=wn[:])

        # persistent SBUF state
        xT = xtp.tile([P, ND, T], BF16)      # x^T, interleaved chunks
        # Q^T/K^T with h on partitions 64:128 and ZEROS on 0:64 (the xbar
        # needs 128-col inputs; the zero half contributes nothing to the
        # h-contraction and costs no extra matmul cycles).
        qzT = qkp.tile([P, T], BF16)
        kzT = qkp.tile([P, T], BF16)
        # staging slots: [:, u, 0:64]=0, [:, u, 64:128]=q (u=0) | k (u=1);
        # one merged PSUM->SBUF copy fills both, rotated i%2.
        qk_stage = [const.tile([P, 2, P], BF16, name=f"qkz{s}")
                    for s in range(2)]
        for t_ in qk_stage:
            nc.gpsimd.memset(t_[:, :, 0:H], 0.0)

        # ---- per-tile pipeline ----
        def tile_work(i):
            g, u = divmod(i, 2)
            # cast f32 -> bf16 into the interleaved (a, j) layout:
            # element d of the tile lands at [a = d % 8, j = d // 8].
            xbt = xbtp.tile([P, ND, P], BF16, tag="xbt", name=f"xbt{i}")
            nc.vector.tensor_copy(
                out=xbt[:].rearrange("p a j -> p j a"), in_=xfs[g][:, u, :])
            # PE transposes: chunk a -> x^T[:, a, tile i]
            px = pxt.tile([P, ND, P], BF16, tag="pxt", name=f"px{i}")
            for a in range(ND):
                nc.tensor.transpose(px[:, a, :], xbt[:, a, :], ident_bf[:])
            nc.vector.tensor_copy(
                out=xT[:, :, i * P:(i + 1) * P], in_=px[:])
            # projection: x^T-stationary, W moving (192 wide)
            ps_p = psmall.tile([P, 3 * H], FP32, tag="small", name=f"psp{i}")
            for a in range(ND):
                nc.tensor.matmul(ps_p[:], xT[:, a, i * P:(i + 1) * P],
                                 w_all[:, a, :],
                                 start=(a == 0), stop=(a == ND - 1))
            # Q,K -> half-zero bf16 stages -> xbar transposes
            qkz = qk_stage[i % 2]
            nc.vector.tensor_copy(out=qkz[:, :, H:P], in_=ps_p[:, 0:P])
            nc.scalar.dma_start(out=qzT[:, i * P:(i + 1) * P],
                                in_=qkz[:, 0, :], transpose=True)
            nc.sync.dma_start(out=kzT[:, i * P:(i + 1) * P],
                              in_=qkz[:, 1, :], transpose=True)
            # V stays t-major
            nc.scalar.copy(out=v_aug[:, i, 0:H], in_=ps_p[:, P:P + H])

        # ---- attention ----
        stores = []

        def diag(b, ki):
            return 4 * b <= ki < 4 * b + 4

        def attention_block(b):
            nk = 4 * b + 4
            qlo = 512 * b
            pairs = [(2 * j, 2 * j + 1) for j in range(nk // 2)]
            ps_o = pout.tile([H + 1, 512], FP32, tag="pout", name=f"pso{b}")

            def s_exp(pr):
                k0, k1 = pr
                w0 = max(0, k0 * P - qlo)
                w1 = max(0, k1 * P - qlo)
                ps = psS.tile([P, 1024], FP32, tag="psS", name=f"psS{b}_{k0}")
                pt = ptp.tile([P, 1024], BF16, tag="pt", name=f"pt{b}_{k0}")
                for ki, w, pos in ((k0, w0, 0), (k1, w1, 512)):
                    nc.tensor.matmul(
                        ps[:, pos + w:pos + 512],
                        kzT[:, ki * P:(ki + 1) * P],
                        qzT[:, qlo + w:qlo + 512],
                        start=True, stop=True)
                if b == 0:
                    # fresh PSUM slots: exp only over written regions
                    for ki, w, pos in ((k0, w0, 0), (k1, w1, 512)):
                        nc.scalar.activation(
                            out=pt[:, pos + w:pos + 512],
                            in_=ps[:, pos + w:pos + 512],
                            func=mybir.ActivationFunctionType.Exp,
                            scale=0.125)
                else:
                    # one wide exp; the [512, 512+w1) gap holds stale
                    # (finite) values from an earlier pair and is never
                    # read by PV.
                    nc.scalar.activation(
                        out=pt[:, w0:1024], in_=ps[:, w0:1024],
                        func=mybir.ActivationFunctionType.Exp,
                        scale=0.125)
                for ki, w, pos in ((k0, w0, 0), (k1, w1, 512)):
                    if diag(b, ki):
                        nc.gpsimd.tensor_mul(pt[:, pos + w:pos + w + P],
                                             pt[:, pos + w:pos + w + P],
                                             tri01[:])
                return pt, w0, w1

            def pv(idx, pr, pt_w):
                k0, k1 = pr
                pt, w0, w1 = pt_w
                for ki, w, pos in ((k0, w0, 0), (k1, w1, 512)):
                    nc.tensor.matmul(
                        ps_o[:, w:512], v_aug[:, ki, 0:H + 1],
                        pt[:, pos + w:pos + 512],
                        start=(idx == 0 and ki == k0),
                        stop=(idx == len(pairs) - 1 and ki == k1))

            pending = s_exp(pairs[0])
            for idx, pr in enumerate(pairs):
                nxt = s_exp(pairs[idx + 1]) if idx + 1 < len(pairs) else None
                pv(idx, pr, pending)
                pending = nxt
            return ps_o

        def out_stage(b, ps_o):
            # Deferred past the next couple of tiles so its PSUM-gated
            # vector ops never head-of-line-block the streaming casts.
            ob = obp.tile([H + 1, 512], BF16, tag="ob", name=f"ob{b}")
            nc.scalar.copy(out=ob[:], in_=ps_o[:])
            pot = psmall.tile([P, 4, VA], BF16, tag="small", name=f"pot{b}")
            for j in range(4):
                nc.tensor.transpose(pot[:, j, 0:H + 1],
                                    ob[:, j * P:(j + 1) * P],
                                    ident_bf[0:H + 1, 0:H + 1])
            rec = recp.tile([P, 4], FP32, tag="rec", name=f"rec{b}")
            nc.vector.reciprocal(rec[:], pot[:, :, H])
            osb = osbp.tile([P, 4, H], BF16, tag="osb", name=f"osb{b}")
            for j in range(4):
                nc.vector.tensor_scalar_mul(osb[:, j, :], pot[:, j, 0:H],
                                            rec[:, j:j + 1])
            stores.append(
                (out_d.rearrange("(b j p) h -> b p j h", p=P, j=4)[b], osb))

        pso = {}
        for i in range(NT):
            tile_work(i)
            if i % 4 == 3:
                pso[i // 4] = attention_block(i // 4)
            if i % 4 == 1 and i > 4:
                out_stage(i // 4 - 1, pso.pop(i // 4 - 1))
        out_stage(3, pso.pop(3))

        for dst, osb in stores:
            nc.sync.dma_start(out=dst, in_=osb[:])


def _run(inputs, trace=False, **kw):
    global _compiled
    if _compiled is None:
        _compiled = _build()
    nc = _compiled
    x = np.ascontiguousarray(inputs["x"], dtype=np.float32)
    wq = np.ascontiguousarray(inputs["Wq"], dtype=np.float32)
    wk = np.ascontiguousarray(inputs["Wk"], dtype=np.float32)
    wv = np.ascontiguousarray(inputs["Wv"], dtype=np.float32)
    in_maps = [
        {"x": np.ascontiguousarray(x[i]), "Wq": wq, "Wk": wk, "Wv": wv}
        for i in range(B)
    ]
    res = run_bass_kernel_spmd(nc, in_maps, core_ids=list(range(B)),
                               trace=trace, **kw)
    out = np.stack(
        [np.asarray(res.results[i]["out"]).astype(np.float32) for i in range(B)],
        axis=0)
    return out, res


def kernel(x, Wq, Wk, Wv):
    out, _ = _run({"x": x, "Wq": Wq, "Wk": Wk, "Wv": Wv})
    return out


# revision 23
# speedup vs baseline: 1.7011x; 1.5207x over previous
"""Single-head causal attention on 8 TRN2 NeuronCores.

Problem: x [8, 2048, 1024] f32, Wq/Wk/Wv [1024, 64] f32.
  q = x @ Wq ; k = x @ Wk ; v = x @ Wv        (per batch)
  out = softmax(causal(q k^T / 8)) @ v        [8, 2048, 64]

Sharding: data-parallel over batch -- core i handles batch element i.
No collectives needed.

Per-core kernel (bf16 compute, f32 accumulate), 128-token-tile pipeline:
  1. W loads use the natural row-contiguous layout (2KB descriptors, no
     sub-512B DMA penalty); the d-contraction is chunked INTERLEAVED
     (chunk a = {d : d = 8p + a}) so the natural layout needs no
     on-chip weight transpose -- gpsimd packs [Wq|Wk|Wv] to bf16.
  2. x streams per 256-token group; each 128-tile is cast f32->bf16 on
     DVE directly into the interleaved layout, transposed on the PE
     (8x [128,128] identity matmuls -> PSUM bf16), and copied to the
     x^T SBUF pool by DVE.
  3. Projections are x-stationary: lhsT = x^T tile-chunk, moving
     rhs = [Wq|Wk|Wv] (192 wide) -> PSUM [t,192] in 8 matmuls/tile
     (1536 PE cycles vs 2048 for the W-stationary form).
  4. Q,K land t-major; one DMA-xbar transpose per tile ([t,128] ->
     [qk,t], 8 ucode tiles = ~112ns of DMA) yields Q^T/K^T rows with
     h on partitions. V stays t-major (what PV wants) and is copied
     into V_aug with a ones column (softmax denominator for free).
  5. Attention per 512-row q-block: S^T[tk,tq] = K^T_tile.T @ Q^T
     (contraction h); exp on ACT in k-tile PAIRS (halves the ~185ns
     per-instruction access-latency overhead); causal diagonal via a
     multiplicative 0/1 bf16 mask on DVE; PV accumulates
     out^T[65,tq] += V_aug.T @ P^T in PSUM, row 64 = denominators.
  6. Output: PSUM -> bf16 SBUF copy (gpsimd), PE-transpose back to
     [tq,65], reciprocal-rescale (DVE recip + gpsimd scale), bf16
     store (f32 upcast happens host-side after gather).

Engine budget: PE ~76k cycles (transposes 16.4k, proj 24.6k, S 16.9k,
PV 16.9k, out 1k) is the critical resource; DMA ~29us (x 23.3 =
roofline, W 2.2, qk-xbar 1.8, stores 1.5); ACT owns exp (~20us); DVE
casts/copies/masks (~22us); gpsimd does the PSUM drains (~11us).
"""

import numpy as np

import concourse.bass as bass
import concourse.tile as tile
from concourse import bacc, mybir
from concourse.bass_utils import run_bass_kernel_spmd

B, T, D, H = 8, 2048, 1024, 64
P = 128            # partitions / tile edge
ND = D // P        # 8 d-chunks (interleaved: chunk a = {d : d = 8p + a})
NT = T // P        # 16 token tiles
NB = T // 512      # 4 q-blocks of 512 rows
VA = 80            # v_aug padded k-tile stride

FP32 = mybir.dt.float32
BF16 = mybir.dt.bfloat16

_compiled = None


def _build():
    nc = bacc.Bacc("TRN2", target_bir_lowering=False, debug=False, num_devices=8)

    x_d = nc.dram_tensor("x", [T, D], FP32, kind="ExternalInput").ap()
    wq_d = nc.dram_tensor("Wq", [D, H], FP32, kind="ExternalInput").ap()
    wk_d = nc.dram_tensor("Wk", [D, H], FP32, kind="ExternalInput").ap()
    wv_d = nc.dram_tensor("Wv", [D, H], FP32, kind="ExternalInput").ap()
    out_d = nc.dram_tensor("out", [T, H], BF16, kind="ExternalOutput").ap()

    with tile.TileContext(nc) as tc:
        _kernel(tc, out_d, x_d, wq_d, wk_d, wv_d)

    nc.compile()
    return nc


def _kernel(tc, out_d, x_d, wq_d, wk_d, wv_d):
    nc = tc.nc
    from contextlib import ExitStack

    ctx = ExitStack()
    with ctx:
        const = ctx.enter_context(tc.tile_pool(name="const", bufs=1))
        wstage = ctx.enter_context(tc.tile_pool(name="wstage", bufs=3))
        xload = ctx.enter_context(tc.tile_pool(name="xload", bufs=4))
        xbtp = ctx.enter_context(tc.tile_pool(name="xbtp", bufs=3))
        xtp = ctx.enter_context(tc.tile_pool(name="xtp", bufs=1))
        qkp = ctx.enter_context(tc.tile_pool(name="qkp", bufs=1))
        qksp = ctx.enter_context(tc.tile_pool(name="qksp", bufs=2))
        vsb = ctx.enter_context(tc.tile_pool(name="vsb", bufs=1))
        ptp = ctx.enter_context(tc.tile_pool(name="ptp", bufs=3))
        obp = ctx.enter_context(tc.tile_pool(name="obp", bufs=2))
        osbp = ctx.enter_context(tc.tile_pool(name="osbp", bufs=2))
        recp = ctx.enter_context(tc.tile_pool(name="recp", bufs=2))
        pxt = ctx.enter_context(tc.tile_pool(name="pxt", bufs=1, space="PSUM"))
        psS = ctx.enter_context(tc.tile_pool(name="psS", bufs=2, space="PSUM"))
        pout = ctx.enter_context(tc.tile_pool(name="pout", bufs=1, space="PSUM"))
        psmall = ctx.enter_context(tc.tile_pool(name="psmall", bufs=2, space="PSUM"))

        # ---- constants ----
        ident_bf = const.tile([P, P], BF16)
        from concourse.masks import make_identity
        make_identity(nc, ident_bf[:])

        # 0/1 upper-triangular (incl. diagonal) bf16 mask in [tk, tq]
        # orientation: valid when tq >= tk (col >= row).
        tri01 = const.tile([P, P], BF16)
        nc.gpsimd.memset(tri01[:], 1.0)
        nc.gpsimd.affine_select(
            out=tri01[:], in_=tri01[:],
            compare_op=mybir.AluOpType.is_ge,
            fill=0.0, base=0,
            pattern=[[1, P]], channel_multiplier=-1)

        # V_aug [t-part, k-tile, 80]: col 64 = 1.0 (denominator row).
        v_aug = vsb.tile([P, NT, VA], BF16)
        nc.gpsimd.memset(v_aug[:, :, H:H + 1], 1.0)

        # ---- weight loads (natural layout; interleaved d-chunking) ----
        # Partition p holds rows d = 8p..8p+7 as one contiguous 2KB
        # descriptor; chunk a of the contraction is the partition-slice
        # [:, a, :], so no weight transpose is ever needed.
        w_all = const.tile([P, ND, 3 * H], BF16)   # [Wq | Wk | Wv] per slot
        wnats = []
        for w_dram, name in ((wq_d, "wq"), (wk_d, "wk"), (wv_d, "wv")):
            wn = wstage.tile([P, ND, H], FP32, tag="wstage", name=f"stg_{name}")
            nc.sync.dma_start(out=wn[:], in_=w_dram.rearrange(
                "(p a) h -> p a h", p=P))
            wnats.append(wn)

        # ---- x loads: 8 groups of 2 tiles ----
        x_r = x_d.rearrange("(g u p) d -> g p u d", p=P, u=2)
        xfs = {}
        for g in range(NT // 2):
            xf = xload.tile([P, 2, D], FP32, tag="xf", name=f"xf{g}")
            nc.sync.dma_start(out=xf[:], in_=x_r[g])
            xfs[g] = xf

        # weight pack (gpsimd): cast + concat into w_all
        for j, wn in enumerate(wnats):
            nc.gpsimd.tensor_copy(out=w_all[:, :, j * H:(j + 1) * H], in_=wn[:])

        # persistent SBUF state
        xT = xtp.tile([P, ND, T], BF16)      # x^T, interleaved chunks
        # Q^T/K^T with h on partitions 64:128 and ZEROS on 0:64 (the xbar
        # needs 128-col inputs; the zero half contributes nothing to the
        # h-contraction and costs no extra matmul cycles).
        qzT = qkp.tile([P, T], BF16)
        kzT = qkp.tile([P, T], BF16)
        # staging slots for 4-tile groups: [:, u, j, 0:64]=0,
        # [:, u, j, 64:128] = q (u=0) | k (u=1) of group-tile j; one merged
        # PSUM->SBUF copy per tile fills both planes, one xbar per plane
        # per GROUP (4 tiles batched -> 8 DMA-transposes total, not 32).
        qk_stage = [const.tile([P, 2, 4, P], BF16, name=f"qkz{s}")
                    for s in range(2)]
        for t_ in qk_stage:
            nc.gpsimd.memset(t_[:, :, :, 0:H], 0.0)

        # ---- per-tile pipeline ----
        def tile_work(i):
            g, u = divmod(i, 2)
            # cast f32 -> bf16 into the interleaved (a, j) layout:
            # element d of the tile lands at [a = d % 8, j = d // 8].
            xbt = xbtp.tile([P, ND, P], BF16, tag="xbt", name=f"xbt{i}")
            nc.vector.tensor_copy(
                out=xbt[:].rearrange("p a j -> p j a"), in_=xfs[g][:, u, :])
            # PE transposes: chunk a -> x^T[:, a, tile i]
            px = pxt.tile([P, ND, P], BF16, tag="pxt", name=f"px{i}")
            for a in range(ND):
                nc.tensor.transpose(px[:, a, :], xbt[:, a, :], ident_bf[:])
            nc.vector.tensor_copy(
                out=xT[:, :, i * P:(i + 1) * P], in_=px[:])
            # projection: x^T-stationary, W moving (192 wide)
            ps_p = psmall.tile([P, 3 * H], FP32, tag="small", name=f"psp{i}")
            for a in range(ND):
                nc.tensor.matmul(ps_p[:], xT[:, a, i * P:(i + 1) * P],
                                 w_all[:, a, :],
                                 start=(a == 0), stop=(a == ND - 1))
            # Q,K -> half-zero bf16 group stage (xbar'd per 4-tile group)
            qkz = qk_stage[(i // 4) % 2]
            nc.vector.tensor_copy(out=qkz[:, :, i % 4, H:P], in_=ps_p[:, 0:P])
            # V stays t-major
            nc.scalar.copy(out=v_aug[:, i, 0:H], in_=ps_p[:, P:P + H])

        def qk_xbars(g):
            # out must stay 3-D [c', j, t']: the middle dim extends the
            # partition dim (transposed row r = j*128 + c').
            qkz = qk_stage[g % 2]
            nc.scalar.dma_start(
                out=qzT[:, g * 512:(g + 1) * 512].rearrange(
                    "p (j t) -> p j t", j=4),
                in_=qkz[:, 0], transpose=True)
            nc.sync.dma_start(
                out=kzT[:, g * 512:(g + 1) * 512].rearrange(
                    "p (j t) -> p j t", j=4),
                in_=qkz[:, 1], transpose=True)

        # ---- attention ----
        stores = []

        def diag(b, ki):
            return 4 * b <= ki < 4 * b + 4

        def attention_block(b):
            nk = 4 * b + 4
            qlo = 512 * b
            pairs = [(2 * j, 2 * j + 1) for j in range(nk // 2)]
            ps_o = pout.tile([H + 1, 512], FP32, tag="pout", name=f"pso{b}")

            def s_exp(pr):
                k0, k1 = pr
                w0 = max(0, k0 * P - qlo)
                w1 = max(0, k1 * P - qlo)
                ps = psS.tile([P, 1024], FP32, tag="psS", name=f"psS{b}_{k0}")
                pt = ptp.tile([P, 1024], BF16, tag="pt", name=f"pt{b}_{k0}")
                for ki, w, pos in ((k0, w0, 0), (k1, w1, 512)):
                    nc.tensor.matmul(
                        ps[:, pos + w:pos + 512],
                        kzT[:, ki * P:(ki + 1) * P],
                        qzT[:, qlo + w:qlo + 512],
                        start=True, stop=True)
                if b == 0:
                    # fresh PSUM slots: exp only over written regions
                    for ki, w, pos in ((k0, w0, 0), (k1, w1, 512)):
                        nc.scalar.activation(
                            out=pt[:, pos + w:pos + 512],
                            in_=ps[:, pos + w:pos + 512],
                            func=mybir.ActivationFunctionType.Exp,
                            scale=0.125)
                else:
                    # one wide exp; the [512, 512+w1) gap holds stale
                    # (finite) values from an earlier pair and is never
                    # read by PV.
                    nc.scalar.activation(
                        out=pt[:, w0:1024], in_=ps[:, w0:1024],
                        func=mybir.ActivationFunctionType.Exp,
                        scale=0.125)
                for ki, w, pos in ((k0, w0, 0), (k1, w1, 512)):
                    if diag(b, ki):
                        nc.gpsimd.tensor_mul(pt[:, pos + w:pos + w + P],
                                             pt[:, pos + w:pos + w + P],
                                             tri01[:])
                return pt, w0, w1

            def pv(idx, pr, pt_w):
                k0, k1 = pr
                pt, w0, w1 = pt_w
                for ki, w, pos in ((k0, w0, 0), (k1, w1, 512)):
                    nc.tensor.matmul(
                        ps_o[:, w:512], v_aug[:, ki, 0:H + 1],
                        pt[:, pos + w:pos + 512],
                        start=(idx == 0 and ki == k0),
                        stop=(idx == len(pairs) - 1 and ki == k1))

            pending = s_exp(pairs[0])
            for idx, pr in enumerate(pairs):
                nxt = s_exp(pairs[idx + 1]) if idx + 1 < len(pairs) else None
                pv(idx, pr, pending)
                pending = nxt
            return ps_o

        def out_stage(b, ps_o):
            # Deferred past the next couple of tiles so its PSUM-gated
            # vector ops never head-of-line-block the streaming casts.
            ob = obp.tile([H + 1, 512], BF16, tag="ob", name=f"ob{b}")
            nc.scalar.copy(out=ob[:], in_=ps_o[:])
            pot = psmall.tile([P, 4, VA], BF16, tag="small", name=f"pot{b}")
            for j in range(4):
                nc.tensor.transpose(pot[:, j, 0:H + 1],
                                    ob[:, j * P:(j + 1) * P],
                                    ident_bf[0:H + 1, 0:H + 1])
            rec = recp.tile([P, 4], FP32, tag="rec", name=f"rec{b}")
            nc.vector.reciprocal(rec[:], pot[:, :, H])
            osb = osbp.tile([P, 4, H], BF16, tag="osb", name=f"osb{b}")
            for j in range(4):
                nc.vector.tensor_scalar_mul(osb[:, j, :], pot[:, j, 0:H],
                                            rec[:, j:j + 1])
            stores.append(
                (out_d.rearrange("(b j p) h -> b p j h", p=P, j=4)[b], osb))

        # Emission tracks data arrival: group xbars fire at each 4th tile;
        # block b's S-work is emitted ~2 tiles later (when its Q^T/K^T can
        # actually exist) so parked PE instructions never head-of-line
        # block ready ones (PE wait queue is only 4 deep); out stages are
        # deferred 2 further tiles.
        pso = {}
        for i in range(NT):
            tile_work(i)
            if i % 4 == 3:
                qk_xbars(i // 4)
            if i >= 5 and (i - 5) % 4 == 0:
                b = (i - 5) // 4
                pso[b] = attention_block(b)
            if i >= 7 and (i - 7) % 4 == 0:
                b = (i - 7) // 4
                out_stage(b, pso.pop(b))
        pso[3] = attention_block(3)
        out_stage(3, pso.pop(3))

        for dst, osb in stores:
            nc.sync.dma_start(out=dst, in_=osb[:])


def _run(inputs, trace=False, **kw):
    global _compiled
    if _compiled is None:
        _compiled = _build()
    nc = _compiled
    x = np.ascontiguousarray(inputs["x"], dtype=np.float32)
    wq = np.ascontiguousarray(inputs["Wq"], dtype=np.float32)
    wk = np.ascontiguousarray(inputs["Wk"], dtype=np.float32)
    wv = np.ascontiguousarray(inputs["Wv"], dtype=np.float32)
    in_maps = [
        {"x": np.ascontiguousarray(x[i]), "Wq": wq, "Wk": wk, "Wv": wv}
        for i in range(B)
    ]
    res = run_bass_kernel_spmd(nc, in_maps, core_ids=list(range(B)),
                               trace=trace, **kw)
    out = np.stack(
        [np.asarray(res.results[i]["out"]).astype(np.float32) for i in range(B)],
        axis=0)
    return out, res


def kernel(x, Wq, Wk, Wv):
    out, _ = _run({"x": x, "Wq": Wq, "Wk": Wk, "Wv": Wv})
    return out
